# revision 7
# baseline (speedup 1.0000x reference)
"""Trainium2 Bass kernel for nn_Attention_5480378269697 (sparse_attention).

Structure (derived from the reference):
  All four attention passes group CONTIGUOUS runs of the flat token axis
  (the reference's reshapes are plain row-major reshapes): pass1/pass4 use
  32 groups of 256 tokens, pass2/pass3 use 16 groups of 512 tokens. The
  softmax is over the CLIP axis (8 clips), so sharding one clip per
  NeuronCore makes everything local except the softmax max/denominator,
  which are 8-core AllReduces.

  Scores use the identity (x@Wq)@(y@Wk)^T == x@(Wq Wk^T)@y^T; G = Wq Wk^T
  is precomputed on host, eliminating the k-projection.

  Precision (validated against the reference numerically): passes 1-2 and
  pass-3 z/s/v run in true fp32 (the clip-softmax is exponentially
  sensitive to score error); pass-3 AV and all of pass 4 run as float32r
  (full-rate fp32-storage matmul).

Layouts: activations live as x^T (channel on partitions, tokens free).
Host pre-transposes frame/bert/masks; output is produced transposed and
un-transposed on host.
"""
import os
import sys
import numpy as np

for _p in ('/opt/trn_rl_repo',):
    if _p not in sys.path:
        sys.path.append(_p)

import concourse.bacc as bacc
import concourse.bass as bass
import concourse.mybir as mybir
import concourse.tile as tile
from concourse import bass_utils

f32 = mybir.dt.float32
f32r = mybir.dt.float32r
ADD = mybir.AluOpType.add
SUB = mybir.AluOpType.subtract
MULT = mybir.AluOpType.mult
MAXOP = mybir.AluOpType.max
EXP = mybir.ActivationFunctionType.Exp
SQRT = mybir.ActivationFunctionType.Sqrt

N_CORES = 8
C = 512
NCH = 4           # channel chunks of 128
T_TOK = 8192      # tokens per clip
NB = 16           # blocks of 512 tokens
BL = 512
EPS = 1e-3
RG = [list(range(N_CORES))]


def build_program():
    nc = bacc.Bacc("TRN2", target_bir_lowering=False, debug=False,
                   enable_asserts=True, num_devices=N_CORES)

    xT = nc.dram_tensor("xT", [C, T_TOK], f32, kind="ExternalInput").ap()
    bertT = nc.dram_tensor("bertT", [C, T_TOK], f32r, kind="ExternalInput").ap()
    m1T = nc.dram_tensor("m1T", [32, 256, 256], f32, kind="ExternalInput").ap()
    m2T = nc.dram_tensor("m2T", [16, 512, 512], f32, kind="ExternalInput").ap()
    m3T = nc.dram_tensor("m3T", [16, 512, 512], f32, kind="ExternalInput").ap()
    G_d = [nc.dram_tensor(f"G{p}", [C, C], f32 if p < 4 else f32r,
                          kind="ExternalInput").ap() for p in range(1, 5)]
    WV_d = [nc.dram_tensor(f"WV{p}", [C, C], f32 if p < 4 else f32r,
                           kind="ExternalInput").ap() for p in range(1, 5)]
    ln_d = {n: nc.dram_tensor(n, [C], f32, kind="ExternalInput").ap()
            for n in ("ln1g", "ln1b", "ln2g", "ln2b")}
    outT = nc.dram_tensor("outT", [C, T_TOK], f32, kind="ExternalOutput").ap()

    with tile.TileContext(nc) as tc:
        with tc.tile_pool(name="wpool", bufs=1) as wpool, \
             tc.tile_pool(name="sb", bufs=1) as sb, \
             tc.tile_pool(name="ps", bufs=1, space="PSUM") as ps, \
             tc.tile_pool(name="dram", bufs=1, space="DRAM") as dram:

            ones_f = wpool.tile([128, 1], f32)
            nc.vector.memset(ones_f[:], 1.0)
            ones_r = wpool.tile([128, 1], f32r)
            nc.vector.tensor_copy(ones_r[:], ones_f[:])
            ln_sb = {}
            for n in ln_d:
                t = wpool.tile([128, NCH], f32, name=f"{n}_sb")
                for ci in range(NCH):
                    nc.sync.dma_start(t[:, ci], ln_d[n][ci * 128:(ci + 1) * 128])
                ln_sb[n] = t

            def load_w(dt_, src, name):
                t = sb.tile([128, NCH, C], dt_, tag="gw", bufs=2, name=name)
                for ci in range(NCH):
                    nc.sync.dma_start(t[:, ci, :], src[ci * 128:(ci + 1) * 128, :])
                return t

            def layer_norm(r_sb, g_t, b_t, writeback, name):
                """r_sb: [128, NCH, BL] (f32r). writeback(ci, tile) stores rows."""
                ps_sum = ps.tile([1, BL], f32, tag="ps_st", bufs=2, name=f"pssum{name}")
                for ci in range(NCH):
                    nc.tensor.matmul(ps_sum[:], lhsT=ones_r[:], rhs=r_sb[:, ci, :],
                                     start=(ci == 0), stop=(ci == NCH - 1))
                ps_sq = ps.tile([1, BL], f32, tag="ps_st", bufs=2, name=f"pssq{name}")
                for ci in range(NCH):
                    rsq = sb.tile([128, BL], f32r, tag="rsq", bufs=2,
                                  name=f"rsq{name}{ci}")
                    nc.vector.tensor_mul(rsq[:], r_sb[:, ci, :], r_sb[:, ci, :])
                    nc.tensor.matmul(ps_sq[:], lhsT=ones_r[:], rhs=rsq[:],
                                     start=(ci == 0), stop=(ci == NCH - 1))
                st_sum = sb.tile([1, BL], f32, tag="st_row", bufs=4, name=f"stsum{name}")
                nc.vector.tensor_copy(st_sum[:], ps_sum[:])
                st_sq = sb.tile([1, BL], f32, tag="st_row", bufs=4, name=f"stsq{name}")
                nc.vector.tensor_copy(st_sq[:], ps_sq[:])
                bc_sum = sb.tile([128, BL], f32, tag="bc", bufs=2, name=f"bcsum{name}")
                nc.gpsimd.partition_broadcast(bc_sum[:], st_sum[:])
                bc_sq = sb.tile([128, BL], f32, tag="bc", bufs=2, name=f"bcsq{name}")
                nc.gpsimd.partition_broadcast(bc_sq[:], st_sq[:])
                mean = sb.tile([128, BL], f32, tag="lnstat", bufs=4, name=f"mean{name}")
                nc.vector.tensor_scalar_mul(mean[:], bc_sum[:], 1.0 / C)
                msq = sb.tile([128, BL], f32, tag="lntmp", bufs=2, name=f"msq{name}")
                nc.vector.tensor_mul(msq[:], mean[:], mean[:])
                var = sb.tile([128, BL], f32, tag="lntmp", bufs=2, name=f"var{name}")
                nc.vector.scalar_tensor_tensor(var[:], bc_sq[:], 1.0 / C, msq[:],
                                               op0=MULT, op1=SUB)
                veps = sb.tile([128, BL], f32, tag="lntmp", bufs=2, name=f"veps{name}")
                nc.vector.tensor_scalar_add(veps[:], var[:], EPS)
                rvar = sb.tile([128, BL], f32, tag="lntmp", bufs=2, name=f"rvar{name}")
                nc.vector.reciprocal(rvar[:], veps[:])
                rstd = sb.tile([128, BL], f32, tag="lnstat", bufs=4, name=f"rstd{name}")
                nc.scalar.activation(rstd[:], rvar[:], SQRT)
                for ci in range(NCH):
                    t1 = sb.tile([128, BL], f32, tag="lnt", bufs=2, name=f"lnt{name}{ci}")
                    nc.vector.tensor_sub(t1[:], r_sb[:, ci, :], mean[:])
                    t2 = sb.tile([128, BL], f32, tag="lnt", bufs=2, name=f"lnu{name}{ci}")
                    nc.vector.tensor_mul(t2[:], t1[:], rstd[:])
                    o = sb.tile([128, BL], f32, tag="lno", bufs=2, name=f"lno{name}{ci}")
                    nc.vector.tensor_scalar(o[:], t2[:],
                                            g_t[:, ci:ci + 1], b_t[:, ci:ci + 1],
                                            op0=MULT, op1=ADD)
                    writeback(ci, o)

            def proj_z(g_w, x, dt_, name):
                z = sb.tile([128, NCH, BL], dt_, tag="z", bufs=2, name=f"z{name}")
                for co in range(NCH):
                    pz = ps.tile([128, BL], f32, tag="psA", bufs=2, name=f"pz{name}{co}")
                    for ci in range(NCH):
                        nc.tensor.matmul(pz[:], lhsT=g_w[:, ci, co * 128:(co + 1) * 128],
                                         rhs=x[:, ci, :],
                                         start=(ci == 0), stop=(ci == NCH - 1))
                    nc.scalar.copy(z[:, co, :], pz[:])
                return z

            def proj_v(wv_w, x, dt_, name):
                v = sb.tile([128, NCH, BL], dt_, tag="v", bufs=2, name=f"v{name}")
                for tk in range(NCH):
                    pv = ps.tile([128, BL], f32, tag="psA", bufs=2, name=f"pv{name}{tk}")
                    for ci in range(NCH):
                        nc.tensor.matmul(pv[:], lhsT=x[:, ci, tk * 128:(tk + 1) * 128],
                                         rhs=wv_w[:, ci, :],
                                         start=(ci == 0), stop=(ci == NCH - 1))
                    nc.scalar.copy(v[:, tk, :], pv[:])
                return v

            def softmax_a(eb, db, a_dt, L, name):
                """AllReduce-add eb -> db, then a = e * (1/d), re-reading e."""
                nc.gpsimd.collective_compute("AllReduce", ADD, replica_groups=RG,
                                             ins=[eb.opt()], outs=[db.opt()])
                a = sb.tile([128, 4, L], a_dt, tag="a", bufs=2, name=f"a{name}")
                dma_e = nc.sync.dma_start if a_dt == f32 else nc.gpsimd.dma_start
                for kk in range(4):
                    dt_ = sb.tile([128, L], f32, tag="dt1", bufs=3, name=f"d{name}{kk}")
                    nc.sync.dma_start(dt_[:], db[:, kk, :])
                    rd = sb.tile([128, L], f32, tag="rd1", bufs=3, name=f"rd{name}{kk}")
                    nc.vector.reciprocal(rd[:], dt_[:])
                    dma_e(a[:, kk, :], eb[:, kk, :])
                    nc.vector.tensor_mul(a[:, kk, :], a[:, kk, :], rd[:])
                return a

            # persistent cross-stage scratch (per block)
            x1s, x2s, ffs = [], [], []

            # ============ STAGE 1 (pass 1, fp32, groups of 256) ============
            g_w = load_w(f32, G_d[0], "g1w")
            wv_w = load_w(f32, WV_d[0], "wv1w")
            for b in range(NB):
                x = sb.tile([128, NCH, BL], f32, tag="x", bufs=2, name=f"x1_{b}")
                for ci in range(NCH):
                    nc.sync.dma_start(x[:, ci, :],
                                      xT[ci * 128:(ci + 1) * 128, b * BL:(b + 1) * BL])
                z = proj_z(g_w, x, f32, f"1_{b}")
                v = proj_v(wv_w, x, f32, f"1_{b}")
                eb = dram.tile([128, 4, 256], f32, tag="eb1", bufs=2, name=f"eb1_{b}")
                for h in range(2):
                    g = 2 * b + h
                    for kc in range(2):
                        kk = 2 * h + kc
                        mk = sb.tile([128, 256], f32, tag="mask1", bufs=3,
                                     name=f"mk1_{b}{kk}")
                        nc.sync.dma_start(mk[:], m1T[g, kc * 128:(kc + 1) * 128, :])
                        pss = ps.tile([128, 256], f32, tag="psS", bufs=2,
                                      name=f"ps1_{b}{kk}")
                        for ci in range(NCH):
                            nc.tensor.matmul(
                                pss[:],
                                lhsT=x[:, ci, h * 256 + kc * 128:h * 256 + (kc + 1) * 128],
                                rhs=z[:, ci, h * 256:(h + 1) * 256],
                                start=(ci == 0), stop=(ci == NCH - 1))
                        sm = sb.tile([128, 256], f32, tag="sm1", bufs=3,
                                     name=f"sm1_{b}{kk}")
                        nc.vector.tensor_add(sm[:], pss[:], mk[:])
                        nc.scalar.activation(sm[:], sm[:], EXP)
                        nc.sync.dma_start(eb[:, kk, :], sm[:])
                db = dram.tile([128, 4, 256], f32, tag="db1", bufs=2,
                               addr_space="Shared", name=f"db1_{b}")
                a = softmax_a(eb, db, f32, 256, f"1_{b}")
                x1 = dram.tile([128, NCH, BL], f32, tag="X1", bufs=NB, name=f"X1_{b}")
                for h in range(2):
                    for co in range(NCH):
                        py = ps.tile([128, 256], f32, tag="psY", bufs=2,
                                     name=f"py1_{b}{h}{co}")
                        for kc in range(2):
                            kk = 2 * h + kc
                            nc.tensor.matmul(py[:],
                                             lhsT=v[:, kk, co * 128:(co + 1) * 128],
                                             rhs=a[:, kk, :],
                                             start=(kc == 0), stop=(kc == 1))
                        yt = sb.tile([128, 256], f32, tag="y", bufs=3,
                                     name=f"y1_{b}{h}{co}")
                        nc.scalar.copy(yt[:], py[:])
                        nc.sync.dma_start(x1[:, co, h * 256:(h + 1) * 256], yt[:])
                x1s.append(x1)

            # ============ STAGES 2 & 3 (fp32 scores, AllReduce max) ============
            for p in (2, 3):
                g_w = load_w(f32, G_d[p - 1], f"g{p}w")
                wv_w = load_w(f32, WV_d[p - 1], f"wv{p}w")
                mT = m2T if p == 2 else m3T
                srcs = x1s if p == 2 else x2s
                av_dt = f32 if p == 2 else f32r
                for b in range(NB):
                    x = sb.tile([128, NCH, BL], f32, tag="x", bufs=2, name=f"x{p}_{b}")
                    for ci in range(NCH):
                        nc.sync.dma_start(x[:, ci, :], srcs[b][:, ci, :])
                    z = proj_z(g_w, x, f32, f"{p}_{b}")
                    v = proj_v(wv_w, x, av_dt, f"{p}_{b}")
                    smb = dram.tile([128, 4, 512], f32, tag="smb", bufs=2,
                                    name=f"smb{p}_{b}")
                    for kc in range(NCH):
                        mk = sb.tile([128, 512], f32, tag="mask2", bufs=3,
                                     name=f"mk{p}_{b}{kc}")
                        nc.sync.dma_start(mk[:], mT[b, kc * 128:(kc + 1) * 128, :])
                        pss = ps.tile([128, 512], f32, tag="psS", bufs=2,
                                      name=f"ps{p}_{b}{kc}")
                        for ci in range(NCH):
                            nc.tensor.matmul(pss[:],
                                             lhsT=x[:, ci, kc * 128:(kc + 1) * 128],
                                             rhs=z[:, ci, :],
                                             start=(ci == 0), stop=(ci == NCH - 1))
                        sm = sb.tile([128, 512], f32, tag="sm2", bufs=3,
                                     name=f"sm{p}_{b}{kc}")
                        nc.vector.tensor_add(sm[:], pss[:], mk[:])
                        nc.sync.dma_start(smb[:, kc, :], sm[:])
                    mb = dram.tile([128, 4, 512], f32, tag="mb", bufs=2,
                                   addr_space="Shared", name=f"mb{p}_{b}")
                    nc.gpsimd.collective_compute("AllReduce", MAXOP, replica_groups=RG,
                                                 ins=[smb.opt()], outs=[mb.opt()])
                    eb = dram.tile([128, 4, 512], f32, tag="eb2", bufs=2,
                                   name=f"eb{p}_{b}")
                    for kc in range(NCH):
                        smr = sb.tile([128, 512], f32, tag="smr", bufs=3,
                                      name=f"smr{p}_{b}{kc}")
                        nc.sync.dma_start(smr[:], smb[:, kc, :])
                        mx = sb.tile([128, 512], f32, tag="mx", bufs=3,
                                     name=f"mx{p}_{b}{kc}")
                        nc.sync.dma_start(mx[:], mb[:, kc, :])
                        nc.vector.tensor_sub(smr[:], smr[:], mx[:])
                        nc.scalar.activation(smr[:], smr[:], EXP)
                        nc.sync.dma_start(eb[:, kc, :], smr[:])
                    db = dram.tile([128, 4, 512], f32, tag="db2", bufs=2,
                                   addr_space="Shared", name=f"db{p}_{b}")
                    a = softmax_a(eb, db, av_dt, 512, f"{p}_{b}")
                    if p == 2:
                        x2 = dram.tile([128, NCH, BL], f32, tag="X2", bufs=NB,
                                       name=f"X2_{b}")
                        for co in range(NCH):
                            py = ps.tile([128, 512], f32, tag="psY", bufs=2,
                                         name=f"py2_{b}{co}")
                            for kc in range(NCH):
                                nc.tensor.matmul(py[:],
                                                 lhsT=v[:, kc, co * 128:(co + 1) * 128],
                                                 rhs=a[:, kc, :],
                                                 start=(kc == 0), stop=(kc == NCH - 1))
                            yt = sb.tile([128, 512], f32, tag="y", bufs=3,
                                         name=f"y2_{b}{co}")
                            nc.scalar.copy(yt[:], py[:])
                            nc.sync.dma_start(x2[:, co, :], yt[:])
                        x2s.append(x2)
                    else:
                        fr = sb.tile([128, NCH, BL], f32, tag="bert", bufs=2,
                                     name=f"fr_{b}")
                        for ci in range(NCH):
                            nc.sync.dma_start(
                                fr[:, ci, :],
                                xT[ci * 128:(ci + 1) * 128, b * BL:(b + 1) * BL])
                        r = sb.tile([128, NCH, BL], f32r, tag="r", bufs=2,
                                    name=f"r3_{b}")
                        for co in range(NCH):
                            py = ps.tile([128, 512], f32, tag="psY", bufs=2,
                                         name=f"py3_{b}{co}")
                            for kc in range(NCH):
                                nc.tensor.matmul(py[:],
                                                 lhsT=v[:, kc, co * 128:(co + 1) * 128],
                                                 rhs=a[:, kc, :],
                                                 start=(kc == 0), stop=(kc == NCH - 1))
                            nc.vector.tensor_add(r[:, co, :], py[:], fr[:, co, :])
                        fft = dram.tile([128, NCH, BL], f32, tag="FF", bufs=NB,
                                        name=f"FF_{b}")
                        layer_norm(r, ln_sb["ln1g"], ln_sb["ln1b"],
                                   lambda ci, o, _t=fft: nc.sync.dma_start(
                                       _t[:, ci, :], o[:]),
                                   name=f"f{b}")
                        ffs.append(fft)

            # ============ STAGE 4 (pass 4, f32r, bert K/V) ============
            g_w = load_w(f32r, G_d[3], "g4w")
            wv_w = load_w(f32r, WV_d[3], "wv4w")
            for b in range(NB):
                ffr = sb.tile([128, NCH, BL], f32r, tag="x", bufs=2, name=f"ffr_{b}")
                for ci in range(NCH):
                    nc.gpsimd.dma_start(ffr[:, ci, :], ffs[b][:, ci, :])
                bt = sb.tile([128, NCH, BL], f32r, tag="bert", bufs=2, name=f"bt_{b}")
                for ci in range(NCH):
                    nc.sync.dma_start(bt[:, ci, :],
                                      bertT[ci * 128:(ci + 1) * 128,
                                            b * BL:(b + 1) * BL])
                z = proj_z(g_w, ffr, f32r, f"4_{b}")
                v = proj_v(wv_w, bt, f32r, f"4_{b}")
                eb = dram.tile([128, 4, 256], f32, tag="eb1", bufs=2, name=f"eb4_{b}")
                for h in range(2):
                    g = 2 * b + h
                    for kc in range(2):
                        kk = 2 * h + kc
                        mk = sb.tile([128, 256], f32, tag="mask1", bufs=3,
                                     name=f"mk4_{b}{kk}")
                        nc.sync.dma_start(mk[:], m1T[g, kc * 128:(kc + 1) * 128, :])
                        pss = ps.tile([128, 256], f32, tag="psS", bufs=2,
                                      name=f"ps4_{b}{kk}")
                        for ci in range(NCH):
                            nc.tensor.matmul(
                                pss[:],
                                lhsT=bt[:, ci, h * 256 + kc * 128:h * 256 + (kc + 1) * 128],
                                rhs=z[:, ci, h * 256:(h + 1) * 256],
                                start=(ci == 0), stop=(ci == NCH - 1))
                        sm = sb.tile([128, 256], f32, tag="sm1", bufs=3,
                                     name=f"sm4_{b}{kk}")
                        nc.vector.tensor_add(sm[:], pss[:], mk[:])
                        nc.scalar.activation(sm[:], sm[:], EXP)
                        nc.sync.dma_start(eb[:, kk, :], sm[:])
                db = dram.tile([128, 4, 256], f32, tag="db1", bufs=2,
                               addr_space="Shared", name=f"db4_{b}")
                a = softmax_a(eb, db, f32r, 256, f"4_{b}")
                r2 = sb.tile([128, NCH, BL], f32r, tag="r", bufs=2, name=f"r4_{b}")
                for h in range(2):
                    for co in range(NCH):
                        py = ps.tile([128, 256], f32, tag="psY", bufs=2,
                                     name=f"py4_{b}{h}{co}")
                        for kc in range(2):
                            kk = 2 * h + kc
                            nc.tensor.matmul(py[:],
                                             lhsT=v[:, kk, co * 128:(co + 1) * 128],
                                             rhs=a[:, kk, :],
                                             start=(kc == 0), stop=(kc == 1))
                        nc.vector.tensor_add(r2[:, co, h * 256:(h + 1) * 256], py[:],
                                             ffr[:, co, h * 256:(h + 1) * 256])
                layer_norm(r2, ln_sb["ln2g"], ln_sb["ln2b"],
                           lambda ci, o, _b=b: nc.sync.dma_start(
                               outT[ci * 128:(ci + 1) * 128,
                                    _b * BL:(_b + 1) * BL], o[:]),
                           name=f"o{b}")

    nc.compile()
    return nc


_CACHE = {}


def _get_program():
    if "nc" not in _CACHE:
        _CACHE["nc"] = build_program()
    return _CACHE["nc"]


def kernel(**inputs):
    nc_prog = _get_program()
    f = np.float32

    def f64mm(a, b):
        return (np.asarray(a).astype(np.float64)
                @ np.asarray(b).astype(np.float64)).astype(f)

    G = [f64mm(inputs['Wq1'], np.asarray(inputs['Wk1']).T),
         f64mm(inputs['Wq2'], np.asarray(inputs['Wk2']).T),
         f64mm(inputs['Wq3'], np.asarray(inputs['Wk3']).T),
         f64mm(inputs['Wwq'], np.asarray(inputs['Wwk']).T)]
    WV = [np.ascontiguousarray(np.asarray(inputs[k], f))
          for k in ('Wv1', 'Wv2', 'Wv3', 'Wwv')]
    shared = {f"G{p + 1}": G[p] for p in range(4)}
    shared.update({f"WV{p + 1}": WV[p] for p in range(4)})
    shared.update({"ln1g": np.asarray(inputs['ln1_g'], f),
                   "ln1b": np.asarray(inputs['ln1_b'], f),
                   "ln2g": np.asarray(inputs['ln2_g'], f),
                   "ln2b": np.asarray(inputs['ln2_b'], f)})

    ff_in = np.asarray(inputs['frame_features'], f)
    bert = np.asarray(inputs['bert_embeddings'], f)
    ncl, t, h, w, c = ff_in.shape
    in_maps = []
    for n in range(ncl):
        m = dict(shared)
        m["xT"] = np.ascontiguousarray(ff_in[n].reshape(T_TOK, C).T)
        m["bertT"] = np.ascontiguousarray(bert[n].reshape(T_TOK, C).T)
        m["m1T"] = np.ascontiguousarray(
            np.asarray(inputs['mask_t'][n], f).transpose(0, 2, 1))
        m["m2T"] = np.ascontiguousarray(
            np.asarray(inputs['mask_h'][n], f).transpose(0, 2, 1))
        m["m3T"] = np.ascontiguousarray(
            np.asarray(inputs['mask_w'][n], f).transpose(0, 2, 1))
        in_maps.append(m)

    res = bass_utils.run_bass_kernel_spmd(
        nc_prog, in_maps, core_ids=list(range(N_CORES)),
        trace=bool(os.environ.get("BASS_KERNEL_TRACE")))
    _CACHE["res"] = res
    out = np.empty((ncl, t, h, w, c), f)
    for n in range(ncl):
        out[n] = res.results[n]["outT"].T.reshape(t, h, w, c)
    return out


if __name__ == "__main__":
    print("building program...")
    import time
    t0 = time.time()
    _get_program()
    print(f"build+compile took {time.time() - t0:.1f}s")


# revision 14
# speedup vs baseline: 1.1149x; 1.1149x over previous
"""Trainium2 Bass kernel for nn_Attention_5480378269697 (sparse_attention).

Structure (derived from the reference):
  All four attention passes group CONTIGUOUS runs of the flat token axis
  (the reference's reshapes are plain row-major reshapes): pass1/pass4 use
  32 groups of 256 tokens, pass2/pass3 use 16 groups of 512 tokens. The
  softmax is over the CLIP axis (8 clips), so sharding one clip per
  NeuronCore makes everything local except the softmax max/denominator,
  which are 8-core AllReduces (batched over 8-block half-stages and
  software-pipelined against the neighbouring halves' compute).

  Scores use the identity (x@Wq)@(y@Wk)^T == x@(Wq Wk^T)@y^T; G = Wq Wk^T
  is precomputed on host, eliminating the k-projection.

  Precision: the clip-softmax is exponentially sensitive to score error,
  so passes 1-3 run all matmuls in split-bf16 ("bf16x2": a@b ~ ah@bh +
  ah@bl + al@bh, fp32 PSUM accumulation, ~2^-17 effective precision) at
  full TensorE rate; pass-3 AV and all of pass 4 run as float32r.
  Weights and the frame input are split hi/lo on the host; activations
  are split on-device as part of the mandatory PSUM->SBUF copies.

Layouts: activations live as x^T (channel on partitions, tokens free).
Host pre-transposes frame/bert/masks; output is produced transposed and
un-transposed on host.
"""
import os
import sys
import numpy as np

for _p in ('/opt/trn_rl_repo',):
    if _p not in sys.path:
        sys.path.append(_p)

import ml_dtypes
import concourse.bacc as bacc
import concourse.bass as bass
import concourse.mybir as mybir
import concourse.tile as tile
from concourse import bass_utils

f32 = mybir.dt.float32
f32r = mybir.dt.float32r
bf16 = mybir.dt.bfloat16
ADD = mybir.AluOpType.add
SUB = mybir.AluOpType.subtract
MULT = mybir.AluOpType.mult
MAXOP = mybir.AluOpType.max
EXP = mybir.ActivationFunctionType.Exp
SQRT = mybir.ActivationFunctionType.Sqrt

N_CORES = 8
C = 512
NCH = 4           # channel chunks of 128
T_TOK = 8192      # tokens per clip
NB = 16           # blocks of 512 tokens
BL = 512
EPS = 1e-3
RG = [list(range(N_CORES))]


def build_program():
    nc = bacc.Bacc("TRN2", target_bir_lowering=False, debug=False,
                   enable_asserts=True, num_devices=N_CORES)

    xT = nc.dram_tensor("xT", [C, T_TOK], f32, kind="ExternalInput").ap()
    xTh = nc.dram_tensor("xTh", [C, T_TOK], bf16, kind="ExternalInput").ap()
    xTl = nc.dram_tensor("xTl", [C, T_TOK], bf16, kind="ExternalInput").ap()
    bertT = nc.dram_tensor("bertT", [C, T_TOK], f32r, kind="ExternalInput").ap()
    m1T = nc.dram_tensor("m1T", [32, 256, 256], f32, kind="ExternalInput").ap()
    m2T = nc.dram_tensor("m2T", [16, 512, 512], f32, kind="ExternalInput").ap()
    m3T = nc.dram_tensor("m3T", [16, 512, 512], f32, kind="ExternalInput").ap()
    Ghl_d = [nc.dram_tensor(f"Ghl{p}", [C, 2, C], bf16, kind="ExternalInput").ap()
             for p in (1, 2, 3)]
    WVhl_d = [nc.dram_tensor(f"WVhl{p}", [C, 2, C], bf16, kind="ExternalInput").ap()
              for p in (1, 2, 3)]
    G4_d = nc.dram_tensor("G4", [C, C], f32r, kind="ExternalInput").ap()
    WV4_d = nc.dram_tensor("WV4", [C, C], f32r, kind="ExternalInput").ap()
    ln_d = {n: nc.dram_tensor(n, [C], f32, kind="ExternalInput").ap()
            for n in ("ln1g", "ln1b", "ln2g", "ln2b")}
    outT = nc.dram_tensor("outT", [C, T_TOK], f32, kind="ExternalOutput").ap()

    HB = 8  # blocks per half-stage (one AllReduce batch)

    with tile.TileContext(nc) as tc:
        with tc.tile_pool(name="wpool", bufs=1) as wpool, \
             tc.tile_pool(name="sb", bufs=1) as sb, \
             tc.tile_pool(name="ps", bufs=1, space="PSUM") as ps, \
             tc.tile_pool(name="dram", bufs=1, space="DRAM") as dram:

            ones_f = wpool.tile([128, 1], f32)
            nc.vector.memset(ones_f[:], 1.0)
            ones_r = wpool.tile([128, 1], f32r)
            nc.vector.tensor_copy(ones_r[:], ones_f[:])
            ln_sb = {}
            for n in ln_d:
                t = wpool.tile([128, NCH], f32, name=f"{n}_sb")
                for ci in range(NCH):
                    nc.sync.dma_start(t[:, ci], ln_d[n][ci * 128:(ci + 1) * 128])
                ln_sb[n] = t

            def load_whl(src, name):
                t = sb.tile([128, NCH, 2, C], bf16, tag="gw", bufs=3, name=name)
                for ci in range(NCH):
                    nc.sync.dma_start(t[:, ci, :, :],
                                      src[ci * 128:(ci + 1) * 128, :, :])
                return t

            def load_w4(src, name):
                t = sb.tile([128, NCH, C], f32r, tag="gw", bufs=3, name=name)
                for ci in range(NCH):
                    nc.sync.dma_start(t[:, ci, :], src[ci * 128:(ci + 1) * 128, :])
                return t

            def load_xhl(srch, srcl, b, name):
                """bf16 hi/lo activation slabs [128, NCH, BL] each."""
                xh = sb.tile([128, NCH, BL], bf16, tag="xh", bufs=3, name=f"{name}h")
                xl = sb.tile([128, NCH, BL], bf16, tag="xl", bufs=3, name=f"{name}l")
                for ci in range(NCH):
                    if srcl is not None:      # global [C, T_TOK] tensors
                        nc.sync.dma_start(xh[:, ci, :],
                                          srch[ci * 128:(ci + 1) * 128,
                                               b * BL:(b + 1) * BL])
                        nc.sync.dma_start(xl[:, ci, :],
                                          srcl[ci * 128:(ci + 1) * 128,
                                               b * BL:(b + 1) * BL])
                    else:                     # packed scratch [128, NCH, 2, BL]
                        nc.sync.dma_start(xh[:, ci, :], srch[:, ci, 0, :])
                        nc.sync.dma_start(xl[:, ci, :], srch[:, ci, 1, :])
                return xh, xl

            def layer_norm(r_sb, g_t, b_t, writeback, name):
                ps_sum = ps.tile([1, BL], f32, tag="psY", bufs=4, name=f"pssum{name}")
                for ci in range(NCH):
                    nc.tensor.matmul(ps_sum[:], lhsT=ones_r[:], rhs=r_sb[:, ci, :],
                                     start=(ci == 0), stop=(ci == NCH - 1))
                ps_sq = ps.tile([1, BL], f32, tag="psY", bufs=4, name=f"pssq{name}")
                for ci in range(NCH):
                    rsq = sb.tile([128, BL], f32r, tag="rsq", bufs=1,
                                  name=f"rsq{name}{ci}")
                    nc.vector.tensor_mul(rsq[:], r_sb[:, ci, :], r_sb[:, ci, :])
                    nc.tensor.matmul(ps_sq[:], lhsT=ones_r[:], rhs=rsq[:],
                                     start=(ci == 0), stop=(ci == NCH - 1))
                st_sum = sb.tile([1, BL], f32, tag="st_row", bufs=2, name=f"stsum{name}")
                nc.vector.tensor_copy(st_sum[:], ps_sum[:])
                st_sq = sb.tile([1, BL], f32, tag="st_row", bufs=2, name=f"stsq{name}")
                nc.vector.tensor_copy(st_sq[:], ps_sq[:])
                bc_sum = sb.tile([128, BL], f32, tag="bc", bufs=2, name=f"bcsum{name}")
                nc.gpsimd.partition_broadcast(bc_sum[:], st_sum[:])
                bc_sq = sb.tile([128, BL], f32, tag="bc", bufs=2, name=f"bcsq{name}")
                nc.gpsimd.partition_broadcast(bc_sq[:], st_sq[:])
                mean = sb.tile([128, BL], f32, tag="lnstat", bufs=2, name=f"mean{name}")
                nc.vector.tensor_scalar_mul(mean[:], bc_sum[:], 1.0 / C)
                msq = sb.tile([128, BL], f32, tag="lntmp", bufs=2, name=f"msq{name}")
                nc.vector.tensor_mul(msq[:], mean[:], mean[:])
                var = sb.tile([128, BL], f32, tag="lntmp", bufs=2, name=f"var{name}")
                nc.vector.scalar_tensor_tensor(var[:], bc_sq[:], 1.0 / C, msq[:],
                                               op0=MULT, op1=SUB)
                veps = sb.tile([128, BL], f32, tag="lntmp", bufs=2, name=f"veps{name}")
                nc.vector.tensor_scalar_add(veps[:], var[:], EPS)
                rvar = sb.tile([128, BL], f32, tag="lntmp", bufs=2, name=f"rvar{name}")
                rvs = sb.tile([128, BL], f32, tag="rds", bufs=2, name=f"rvs{name}")
                nc.vector.reciprocal_approx_accurate(rvar[:], veps[:], rvs[:])
                rstd = sb.tile([128, BL], f32, tag="lnstat", bufs=2, name=f"rstd{name}")
                nc.scalar.activation(rstd[:], rvar[:], SQRT)
                for ci in range(NCH):
                    t1 = sb.tile([128, BL], f32, tag="lnt", bufs=2, name=f"lnt{name}{ci}")
                    nc.vector.tensor_sub(t1[:], r_sb[:, ci, :], mean[:])
                    nc.vector.tensor_mul(t1[:], t1[:], rstd[:])
                    o = sb.tile([128, BL], f32, tag="lno", bufs=2, name=f"lno{name}{ci}")
                    nc.vector.tensor_scalar(o[:], t1[:],
                                            g_t[:, ci:ci + 1], b_t[:, ci:ci + 1],
                                            op0=MULT, op1=ADD)
                    writeback(ci, o)

            def proj_z_x2(g_w, xh, xl, name):
                """z = G^T x as packed hi/lo bf16 [128, NCH, 2, BL]."""
                z = sb.tile([128, NCH, 2, BL], bf16, tag="z", bufs=2, name=f"z{name}")
                for co in range(NCH):
                    pz = ps.tile([128, BL], f32, tag="psA", bufs=2, name=f"pz{name}{co}")
                    n = 0
                    for ci in range(NCH):
                        for wsel, xx in ((0, xh), (0, xl), (1, xh)):
                            nc.tensor.matmul(
                                pz[:], lhsT=g_w[:, ci, wsel, co * 128:(co + 1) * 128],
                                rhs=xx[:, ci, :],
                                start=(n == 0), stop=(n == 3 * NCH - 1))
                            n += 1
                    nc.vector.tensor_copy(z[:, co, 0, :], pz[:])
                    nc.vector.tensor_sub(z[:, co, 1, :], pz[:], z[:, co, 0, :])
                return z

            def proj_v_x2(wv_w, xh, xl, out_dt, name):
                """v = x@Wv; out packed bf16 hi/lo or single f32r."""
                if out_dt == bf16:
                    v = sb.tile([128, NCH, 2, BL], bf16, tag="v", bufs=2,
                                name=f"v{name}")
                else:
                    v = sb.tile([128, NCH, BL], f32r, tag="v", bufs=2, name=f"v{name}")
                for tk in range(NCH):
                    pv = ps.tile([128, BL], f32, tag="psA", bufs=2, name=f"pv{name}{tk}")
                    n = 0
                    for ci in range(NCH):
                        sl = slice(tk * 128, (tk + 1) * 128)
                        for wsel, xx in ((0, xh), (1, xh), (0, xl)):
                            nc.tensor.matmul(pv[:], lhsT=xx[:, ci, sl],
                                             rhs=wv_w[:, ci, wsel, :],
                                             start=(n == 0), stop=(n == 3 * NCH - 1))
                            n += 1
                    if out_dt == bf16:
                        nc.vector.tensor_copy(v[:, tk, 0, :], pv[:])
                        nc.vector.tensor_sub(v[:, tk, 1, :], pv[:], v[:, tk, 0, :])
                    else:
                        nc.vector.tensor_copy(v[:, tk, :], pv[:])
                return v

            def s_psum(xh, xl, z, koff, kc, qoff, L, name):
                """scores psum: s^T[k-chunk, q-range] via 3 split matmuls x NCH."""
                pss = ps.tile([128, L], f32, tag="psS", bufs=2, name=f"ps{name}")
                sl = slice(koff + kc * 128, koff + (kc + 1) * 128)
                n = 0
                for ci in range(NCH):
                    for xx, zsel in ((xh, 0), (xh, 1), (xl, 0)):
                        nc.tensor.matmul(pss[:], lhsT=xx[:, ci, sl],
                                         rhs=z[:, ci, zsel, qoff:qoff + L],
                                         start=(n == 0), stop=(n == 3 * NCH - 1))
                        n += 1
                return pss

            def softmax_a_kk(eb, db, j, kk, L, a_dt, name):
                """per-k-chunk a = e * 1/d; returns bf16 hi/lo tile or f32r tile."""
                dt_ = sb.tile([128, L], f32, tag="dt1", bufs=2, name=f"d{name}{kk}")
                nc.scalar.dma_start(dt_[:], db[:, j, kk, :])
                rd = sb.tile([128, L], f32, tag="rd1", bufs=2, name=f"rd{name}{kk}")
                rs = sb.tile([128, L], f32, tag="rds", bufs=2, name=f"rs{name}{kk}")
                nc.vector.reciprocal_approx_accurate(rd[:], dt_[:], rs[:])
                if a_dt == bf16:
                    af = sb.tile([128, L], f32, tag="af", bufs=2, name=f"af{name}{kk}")
                    nc.scalar.dma_start(af[:], eb[:, j, kk, :])
                    nc.vector.tensor_mul(af[:], af[:], rd[:])
                    ah = sb.tile([128, 2, L], bf16, tag="ak", bufs=2,
                                 name=f"ah{name}{kk}")
                    nc.vector.tensor_copy(ah[:, 0, :], af[:])
                    nc.vector.tensor_sub(ah[:, 1, :], af[:], ah[:, 0, :])
                    return ah
                ar = sb.tile([128, L], f32r, tag="ak", bufs=2, name=f"ar{name}{kk}")
                nc.gpsimd.dma_start(ar[:], eb[:, j, kk, :])
                nc.vector.tensor_mul(ar[:], ar[:], rd[:])
                return ar

            def av_x2(v, eb, db, j, L, sink, name, kks):
                pys = [ps.tile([128, L], f32, tag="psY", bufs=4,
                               name=f"py{name}{co}") for co in range(NCH)]
                for i, kk in enumerate(kks):
                    ah = softmax_a_kk(eb, db, j, kk, L, bf16, name)
                    for co in range(NCH):
                        for m, (vsel, asel) in enumerate(((0, 0), (0, 1), (1, 0))):
                            nc.tensor.matmul(
                                pys[co][:],
                                lhsT=v[:, kk, vsel, co * 128:(co + 1) * 128],
                                rhs=ah[:, asel, :],
                                start=(i == 0 and m == 0),
                                stop=(i == len(kks) - 1 and m == 2))
                for co in range(NCH):
                    sink(co, pys[co])

            def av_f32r(v, eb, db, j, L, sink, name, kks):
                pys = [ps.tile([128, L], f32, tag="psY", bufs=4,
                               name=f"py{name}{co}") for co in range(NCH)]
                for i, kk in enumerate(kks):
                    ar = softmax_a_kk(eb, db, j, kk, L, f32r, name)
                    for co in range(NCH):
                        nc.tensor.matmul(pys[co][:],
                                         lhsT=v[:, kk, co * 128:(co + 1) * 128],
                                         rhs=ar[:], start=(i == 0),
                                         stop=(i == len(kks) - 1))
                for co in range(NCH):
                    sink(co, pys[co])

            def scores_small_x2(xh, xl, z, b, j, eb, mT_, name):
                for h in range(2):
                    g = 2 * b + h
                    for kc in range(2):
                        kk = 2 * h + kc
                        mk = sb.tile([128, 256], f32, tag="mask1", bufs=2,
                                     name=f"mk{name}{kk}")
                        nc.sync.dma_start(mk[:], mT_[g, kc * 128:(kc + 1) * 128, :])
                        pss = s_psum(xh, xl, z, h * 256, kc, h * 256, 256,
                                     f"{name}{kk}")
                        sm = sb.tile([128, 256], f32, tag="sm1", bufs=2,
                                     name=f"sm{name}{kk}")
                        nc.vector.tensor_add(sm[:], pss[:], mk[:])
                        nc.scalar.activation(sm[:], sm[:], EXP)
                        nc.sync.dma_start(eb[:, j, kk, :], sm[:])

            x1s, x2s, ffs = [], [], []

            def emitA(s, hf):
                blocks = range(hf * HB, (hf + 1) * HB)
                st = {"s": s, "hf": hf, "blocks": blocks}
                if s == 1:
                    g_w = weights[s][0]
                    eb = dram.tile([128, HB, 4, 256], f32, tag="ebH1", bufs=2,
                                   name=f"eb1_{hf}")
                    for j, b in enumerate(blocks):
                        xh, xl = load_xhl(xTh, xTl, b, f"x1_{b}")
                        z = proj_z_x2(g_w, xh, xl, f"1_{b}")
                        scores_small_x2(xh, xl, z, b, j, eb, m1T, f"1_{b}")
                    st["eb"] = eb
                elif s == 4:
                    g_w = weights[s][0]
                    eb = dram.tile([128, HB, 4, 256], f32, tag="ebH1", bufs=2,
                                   name=f"eb4_{hf}")
                    for j, b in enumerate(blocks):
                        ffr = sb.tile([128, NCH, BL], f32r, tag="bert", bufs=3,
                                      name=f"ffr_{b}")
                        for ci in range(NCH):
                            nc.gpsimd.dma_start(ffr[:, ci, :], ffs[b][:, ci, :])
                        bt = sb.tile([128, NCH, BL], f32r, tag="bert", bufs=3,
                                     name=f"bt_{b}")
                        for ci in range(NCH):
                            nc.sync.dma_start(bt[:, ci, :],
                                              bertT[ci * 128:(ci + 1) * 128,
                                                    b * BL:(b + 1) * BL])
                        z = sb.tile([128, NCH, BL], f32r, tag="z", bufs=2,
                                    name=f"z4_{b}")
                        for co in range(NCH):
                            pz = ps.tile([128, BL], f32, tag="psA", bufs=2,
                                         name=f"pz4_{b}{co}")
                            for ci in range(NCH):
                                nc.tensor.matmul(
                                    pz[:], lhsT=g_w[:, ci, co * 128:(co + 1) * 128],
                                    rhs=ffr[:, ci, :],
                                    start=(ci == 0), stop=(ci == NCH - 1))
                            nc.vector.tensor_copy(z[:, co, :], pz[:])
                        for h in range(2):
                            g = 2 * b + h
                            for kc in range(2):
                                kk = 2 * h + kc
                                mk = sb.tile([128, 256], f32, tag="mask1", bufs=2,
                                             name=f"mk4_{b}{kk}")
                                nc.sync.dma_start(
                                    mk[:], m1T[g, kc * 128:(kc + 1) * 128, :])
                                pss = ps.tile([128, 256], f32, tag="psS", bufs=2,
                                              name=f"ps4_{b}{kk}")
                                sl = slice(h * 256 + kc * 128,
                                           h * 256 + (kc + 1) * 128)
                                for ci in range(NCH):
                                    nc.tensor.matmul(
                                        pss[:], lhsT=bt[:, ci, sl],
                                        rhs=z[:, ci, h * 256:(h + 1) * 256],
                                        start=(ci == 0), stop=(ci == NCH - 1))
                                sm = sb.tile([128, 256], f32, tag="sm1", bufs=2,
                                             name=f"sm4_{b}{kk}")
                                nc.vector.tensor_add(sm[:], pss[:], mk[:])
                                nc.scalar.activation(sm[:], sm[:], EXP)
                                nc.sync.dma_start(eb[:, j, kk, :], sm[:])
                    st["eb"] = eb
                else:
                    g_w = weights[s][0]
                    mT = m2T if s == 2 else m3T
                    srcs = x1s if s == 2 else x2s
                    smb = dram.tile([128, HB, 4, 512], f32, tag="smbH", bufs=2,
                                    name=f"smb{s}_{hf}")
                    for j, b in enumerate(blocks):
                        xh, xl = load_xhl(srcs[b], None, b, f"x{s}_{b}")
                        z = proj_z_x2(g_w, xh, xl, f"{s}_{b}")
                        for kc in range(NCH):
                            mk = sb.tile([128, 512], f32, tag="mask2", bufs=2,
                                         name=f"mk{s}_{b}{kc}")
                            nc.sync.dma_start(mk[:],
                                              mT[b, kc * 128:(kc + 1) * 128, :])
                            pss = s_psum(xh, xl, z, 0, kc, 0, 512, f"{s}_{b}{kc}")
                            sm = sb.tile([128, 512], f32, tag="sm2", bufs=2,
                                         name=f"sm{s}_{b}{kc}")
                            nc.vector.tensor_add(sm[:], pss[:], mk[:])
                            nc.sync.dma_start(smb[:, j, kc, :], sm[:])
                    st["smb"] = smb
                return st

            def emitMid(st):
                s, hf = st["s"], st["hf"]
                if s in (1, 4):
                    db = dram.tile([128, HB, 4, 256], f32, tag="dbH1", bufs=2,
                                   addr_space="Shared", name=f"db{s}_{hf}")
                    nc.gpsimd.collective_compute("AllReduce", ADD, replica_groups=RG,
                                                 ins=[st["eb"].opt()], outs=[db.opt()])
                    st["db"] = db
                    return
                smb = st["smb"]
                mb = dram.tile([128, HB, 4, 512], f32, tag="mbH", bufs=2,
                               addr_space="Shared", name=f"mb{s}_{hf}")
                nc.gpsimd.collective_compute("AllReduce", MAXOP, replica_groups=RG,
                                             ins=[smb.opt()], outs=[mb.opt()])
                eb = dram.tile([128, HB, 4, 512], f32, tag="ebH", bufs=2,
                               name=f"eb{s}_{hf}")
                for j, b in enumerate(st["blocks"]):
                    for kc in range(NCH):
                        smr = sb.tile([128, 512], f32, tag="smr", bufs=2,
                                      name=f"smr{s}_{b}{kc}")
                        nc.scalar.dma_start(smr[:], smb[:, j, kc, :])
                        mx = sb.tile([128, 512], f32, tag="mx", bufs=2,
                                     name=f"mx{s}_{b}{kc}")
                        nc.scalar.dma_start(mx[:], mb[:, j, kc, :])
                        nc.vector.tensor_sub(smr[:], smr[:], mx[:])
                        nc.scalar.activation(smr[:], smr[:], EXP)
                        nc.scalar.dma_start(eb[:, j, kc, :], smr[:])
                db = dram.tile([128, HB, 4, 512], f32, tag="dbH", bufs=2,
                               addr_space="Shared", name=f"db{s}_{hf}")
                nc.gpsimd.collective_compute("AllReduce", ADD, replica_groups=RG,
                                             ins=[eb.opt()], outs=[db.opt()])
                st["eb"], st["db"] = eb, db

            def emitC(st):
                s, hf = st["s"], st["hf"]
                eb, db = st["eb"], st["db"]
                wv_w = weights[s][1]
                if s == 1:
                    for j, b in enumerate(st["blocks"]):
                        xh, xl = load_xhl(xTh, xTl, b, f"xv1_{b}")
                        v = proj_v_x2(wv_w, xh, xl, bf16, f"1_{b}")
                        x1 = dram.tile([128, NCH, 2, BL], bf16, tag="X1", bufs=NB,
                                       name=f"X1_{b}")
                        for h in range(2):
                            def sink1(co, py, _x1=x1, _b=b, _h=h):
                                sl = slice(_h * 256, (_h + 1) * 256)
                                yh = sb.tile([128, 256], bf16, tag="y", bufs=4,
                                             name=f"yh1_{_b}{_h}{co}")
                                nc.vector.tensor_copy(yh[:], py[:])
                                yl = sb.tile([128, 256], bf16, tag="y", bufs=4,
                                             name=f"yl1_{_b}{_h}{co}")
                                nc.vector.tensor_sub(yl[:], py[:], yh[:])
                                nc.sync.dma_start(_x1[:, co, 0, sl], yh[:])
                                nc.sync.dma_start(_x1[:, co, 1, sl], yl[:])
                            av_x2(v, eb, db, j, 256, sink1, f"1_{b}{h}",
                                  kks=[2 * h, 2 * h + 1])
                        x1s.append(x1)
                elif s == 4:
                    for j, b in enumerate(st["blocks"]):
                        bt2 = sb.tile([128, NCH, BL], f32r, tag="bert", bufs=3,
                                      name=f"bt2_{b}")
                        for ci in range(NCH):
                            nc.sync.dma_start(bt2[:, ci, :],
                                              bertT[ci * 128:(ci + 1) * 128,
                                                    b * BL:(b + 1) * BL])
                        ffr2 = sb.tile([128, NCH, BL], f32r, tag="bert", bufs=3,
                                       name=f"ffr2_{b}")
                        for ci in range(NCH):
                            nc.gpsimd.dma_start(ffr2[:, ci, :], ffs[b][:, ci, :])
                        v = sb.tile([128, NCH, BL], f32r, tag="v", bufs=2,
                                    name=f"v4_{b}")
                        for tk in range(NCH):
                            pv = ps.tile([128, BL], f32, tag="psA", bufs=2,
                                         name=f"pv4_{b}{tk}")
                            for ci in range(NCH):
                                nc.tensor.matmul(
                                    pv[:],
                                    lhsT=bt2[:, ci, tk * 128:(tk + 1) * 128],
                                    rhs=wv_w[:, ci, :],
                                    start=(ci == 0), stop=(ci == NCH - 1))
                            nc.vector.tensor_copy(v[:, tk, :], pv[:])
                        r2 = sb.tile([128, NCH, BL], f32r, tag="r", bufs=2,
                                     name=f"r4_{b}")
                        for h in range(2):
                            def sink4(co, py, _r2=r2, _ffr=ffr2, _h=h):
                                sl = slice(_h * 256, (_h + 1) * 256)
                                nc.vector.tensor_add(_r2[:, co, sl], py[:],
                                                     _ffr[:, co, sl])
                            av_f32r(v, eb, db, j, 256, sink4, f"4_{b}{h}",
                                    kks=[2 * h, 2 * h + 1])
                        layer_norm(r2, ln_sb["ln2g"], ln_sb["ln2b"],
                                   lambda ci, o, _b=b: nc.sync.dma_start(
                                       outT[ci * 128:(ci + 1) * 128,
                                            _b * BL:(_b + 1) * BL], o[:]),
                                   name=f"o{b}")
                else:
                    srcs = x1s if s == 2 else x2s
                    for j, b in enumerate(st["blocks"]):
                        xh, xl = load_xhl(srcs[b], None, b, f"xv{s}_{b}")
                        v = proj_v_x2(wv_w, xh, xl,
                                      bf16 if s == 2 else f32r, f"{s}_{b}")
                        if s == 2:
                            x2 = dram.tile([128, NCH, 2, BL], bf16, tag="X2",
                                           bufs=NB, name=f"X2_{b}")

                            def sink2(co, py, _x2=x2, _b=b):
                                yh = sb.tile([128, 512], bf16, tag="y", bufs=4,
                                             name=f"yh2_{_b}{co}")
                                nc.vector.tensor_copy(yh[:], py[:])
                                yl = sb.tile([128, 512], bf16, tag="y", bufs=4,
                                             name=f"yl2_{_b}{co}")
                                nc.vector.tensor_sub(yl[:], py[:], yh[:])
                                nc.sync.dma_start(_x2[:, co, 0, :], yh[:])
                                nc.sync.dma_start(_x2[:, co, 1, :], yl[:])
                            av_x2(v, eb, db, j, 512, sink2, f"2_{b}",
                                  kks=[0, 1, 2, 3])
                            x2s.append(x2)
                        else:
                            r = sb.tile([128, NCH, BL], f32r, tag="r", bufs=2,
                                        name=f"r3_{b}")

                            def sink3(co, py, _r=r, _b=b):
                                fr = sb.tile([128, 512], f32, tag="frr", bufs=2,
                                             name=f"fr_{_b}{co}")
                                nc.sync.dma_start(
                                    fr[:], xT[co * 128:(co + 1) * 128,
                                              _b * BL:(_b + 1) * BL])
                                nc.vector.tensor_add(_r[:, co, :], py[:], fr[:])
                            av_f32r(v, eb, db, j, 512, sink3, f"3_{b}",
                                    kks=[0, 1, 2, 3])
                            fft = dram.tile([128, NCH, BL], f32, tag="FF", bufs=NB,
                                            name=f"FF_{b}")
                            layer_norm(r, ln_sb["ln1g"], ln_sb["ln1b"],
                                       lambda ci, o, _t=fft: nc.sync.dma_start(
                                           _t[:, ci, :], o[:]),
                                       name=f"f{b}")
                            ffs.append(fft)

            weights = {}
            halves = [(s, hf) for s in (1, 2, 3, 4) for hf in range(NB // HB)]
            pending = None
            for s, hf in halves:
                if hf == 0:
                    if s < 4:
                        weights[s] = (load_whl(Ghl_d[s - 1], f"g{s}w"),
                                      load_whl(WVhl_d[s - 1], f"wv{s}w"))
                    else:
                        weights[s] = (load_w4(G4_d, "g4w"), load_w4(WV4_d, "wv4w"))
                st = emitA(s, hf)
                emitMid(st)
                if pending is not None:
                    emitC(pending)
                pending = st
            emitC(pending)

    nc.compile()
    return nc


_CACHE = {}


def _get_program():
    if "nc" not in _CACHE:
        _CACHE["nc"] = build_program()
    return _CACHE["nc"]


def _split_hl(a):
    hi = a.astype(ml_dtypes.bfloat16)
    lo = (a - hi.astype(np.float32)).astype(ml_dtypes.bfloat16)
    return hi, lo


def _pack_whl(w):
    hi, lo = _split_hl(np.asarray(w, np.float32))
    return np.ascontiguousarray(np.stack([hi, lo], axis=1))


def kernel(**inputs):
    nc_prog = _get_program()
    f = np.float32

    def f64mm(a, b):
        return (np.asarray(a).astype(np.float64)
                @ np.asarray(b).astype(np.float64)).astype(f)

    G = [f64mm(inputs['Wq1'], np.asarray(inputs['Wk1']).T),
         f64mm(inputs['Wq2'], np.asarray(inputs['Wk2']).T),
         f64mm(inputs['Wq3'], np.asarray(inputs['Wk3']).T),
         f64mm(inputs['Wwq'], np.asarray(inputs['Wwk']).T)]
    shared = {}
    for p in (1, 2, 3):
        shared[f"Ghl{p}"] = _pack_whl(G[p - 1])
        shared[f"WVhl{p}"] = _pack_whl(inputs[('Wv1', 'Wv2', 'Wv3')[p - 1]])
    shared["G4"] = G[3]
    shared["WV4"] = np.ascontiguousarray(np.asarray(inputs['Wwv'], f))
    shared.update({"ln1g": np.asarray(inputs['ln1_g'], f),
                   "ln1b": np.asarray(inputs['ln1_b'], f),
                   "ln2g": np.asarray(inputs['ln2_g'], f),
                   "ln2b": np.asarray(inputs['ln2_b'], f)})

    ff_in = np.asarray(inputs['frame_features'], f)
    bert = np.asarray(inputs['bert_embeddings'], f)
    ncl, t, h, w, c = ff_in.shape
    in_maps = []
    for n in range(ncl):
        m = dict(shared)
        xTn = np.ascontiguousarray(ff_in[n].reshape(T_TOK, C).T)
        m["xT"] = xTn
        hi, lo = _split_hl(xTn)
        m["xTh"] = np.ascontiguousarray(hi)
        m["xTl"] = np.ascontiguousarray(lo)
        m["bertT"] = np.ascontiguousarray(bert[n].reshape(T_TOK, C).T)
        m["m1T"] = np.ascontiguousarray(
            np.asarray(inputs['mask_t'][n], f).transpose(0, 2, 1))
        m["m2T"] = np.ascontiguousarray(
            np.asarray(inputs['mask_h'][n], f).transpose(0, 2, 1))
        m["m3T"] = np.ascontiguousarray(
            np.asarray(inputs['mask_w'][n], f).transpose(0, 2, 1))
        in_maps.append(m)

    res = bass_utils.run_bass_kernel_spmd(
        nc_prog, in_maps, core_ids=list(range(N_CORES)),
        trace=bool(os.environ.get("BASS_KERNEL_TRACE")))
    _CACHE["res"] = res
    out = np.empty((ncl, t, h, w, c), f)
    for n in range(ncl):
        out[n] = res.results[n]["outT"].T.reshape(t, h, w, c)
    return out


if __name__ == "__main__":
    print("building program...")
    import time
    t0 = time.time()
    _get_program()
    print(f"build+compile took {time.time() - t0:.1f}s")
